# revision 29
# baseline (speedup 1.0000x reference)
"""BERT attention (QKV proj + SDPA) sharded over 8 trn2 NeuronCores by head.

Problem: hidden_states [2, 2048, 1024], 16 heads x 64 dim, fp32.
Sharding: 2 heads per core (tensor-parallel on Q/K/V weight columns).

Per-core device kernel (matmul operands bf16, accumulation fp32):
  inputs:  xt  [1024, 4096]  X^T (host-pretransposed, bf16, same on all cores)
           wq/wk/wv [1024, 128]  weight column slice for this core's 2 heads
           bq/bk/bv [128, 1]     bias slice (f32)
  output:  out [4096, 128] f32   context for the 2 heads (token-major)

Dataflow per batch:
  1. QT/KT/VT [c=128, t] = W.T @ X.T (contraction over hidden), bias added
     on DVE during PSUM->SBUF copy.
  2. V' [k, 65] per head via PE-transpose of VT; col 64 = ones (row sums).
  3. Scores TRANSPOSED: ST[k, q] bf16 so softmax-exp output PT[k, q] feeds
     P@V as the moving operand with no transposes:
     ctxT[d|sum, q] = sum_k V'[k, 65].T @ PT[k, q]. Heads at partition
     bases 0/64 pack the d=64-contraction score matmuls into disjoint PE
     row groups (concurrent). exp has no max-subtraction (scores ~ N(0,1));
     the 1/8 scale is folded into the ACT op.
  4. Normalize: PE-transpose ctxT chunks to [q, 65]; per-partition
     reciprocal of the sums column; tensor_scalar multiply; DMA out.

The attention loop is ACT(exp)-bound; projection matmul groups and V'
transposes for later chunks are emitted as "fillers" inside the kt loop so
the PE does them under the exp shadow. PSUM: scores 2x1 bank (bf16),
ctx accumulators 4x1, normalize/proj 2x1.
"""

import numpy as np
import ml_dtypes

B, S, HID = 2, 2048, 1024
T = B * S
N_CORES = 8
P = 128
D = 64
HK = HID // P  # hidden-dim chunks

BF = ml_dtypes.bfloat16

_CACHED = {}


def _build():
    from collections import deque

    import concourse.bass as bass
    from concourse import bacc
    import concourse.tile as tile
    import concourse.mybir as mybir
    from concourse.bass import ts, ds
    from concourse.masks import make_identity

    bf16 = mybir.dt.bfloat16
    f32 = mybir.dt.float32
    Exp = mybir.ActivationFunctionType.Exp

    nc = bacc.Bacc(trn_type="TRN2", target_bir_lowering=False, debug=False)

    xt = nc.dram_tensor("xt", [HID, T], bf16, kind="ExternalInput").ap()
    wq = nc.dram_tensor("wq", [HID, P], bf16, kind="ExternalInput").ap()
    wk = nc.dram_tensor("wk", [HID, P], bf16, kind="ExternalInput").ap()
    wv = nc.dram_tensor("wv", [HID, P], bf16, kind="ExternalInput").ap()
    bias = nc.dram_tensor("bias", [P, 3], f32, kind="ExternalInput").ap()
    out = nc.dram_tensor("out", [T, P], f32, kind="ExternalOutput").ap()

    with tile.TileContext(nc) as tc:
        with (
            tc.tile_pool(name="const", bufs=1) as cpool,
            tc.tile_pool(name="xtp", bufs=1) as xtpool,
            tc.tile_pool(name="qkv", bufs=1) as qkvpool,
            tc.tile_pool(name="pt", bufs=1) as ptpool,
            tc.tile_pool(name="stg", bufs=2) as stgpool,
            tc.tile_pool(name="small", bufs=4) as smallpool,
            tc.tile_pool(name="ot", bufs=2) as otpool,
            tc.tile_pool(name="ps", bufs=2, space="PSUM") as psp,
        ):
            # X^T half-buffer: holds one batch's tokens; batch 1 reloads it
            # (all batch-0 projections are emitted before the reload DMAs).
            # One fused DMA per 512-token quarter (HWDGE issue is ~625 ns per
            # dma_start, so fewer+bigger transfers shorten the critical path).
            xt_sb = xtpool.tile([P, HK, S], bf16, tag="xt")
            xtp = xt.rearrange("(a p) t -> p a t", p=P)
            w_sbs = []
            bias_sb = cpool.tile([P, 3], f32, tag="bias")
            b_sbs = [bias_sb[:, i : i + 1] for i in range(3)]
            for i, name in enumerate(("q", "k", "v")):
                w_sbs.append(
                    cpool.tile([P, HK, P], bf16, tag=f"w{name}", name=f"w{name}sb")
                )
            # DMA arrival order matched to first-consumption order so the PE
            # never idles mid-startup (transfers serialize on the DMA fabric)
            nc.sync.dma_start(xt_sb[:, :, 0:512], xtp[:, :, 0:512])
            nc.sync.dma_start(bias_sb, bias)
            nc.sync.dma_start(w_sbs[0], wq.rearrange("(a p) c -> p a c", p=P))
            nc.sync.dma_start(xt_sb[:, :, ts(1, 512)], xtp[:, :, ts(1, 512)])
            nc.sync.dma_start(w_sbs[1], wk.rearrange("(a p) c -> p a c", p=P))
            nc.sync.dma_start(w_sbs[2], wv.rearrange("(a p) c -> p a c", p=P))
            for quarter in range(2, 4):
                nc.sync.dma_start(
                    xt_sb[:, :, ts(quarter, 512)], xtp[:, :, ts(quarter, 512)]
                )

            ident_bf = cpool.tile([P, P], bf16, tag="identb")
            make_identity(nc, ident_bf)
            ident_f = cpool.tile([P, P], f32, tag="identf")
            make_identity(nc, ident_f)

            qt_sb = qkvpool.tile([P, T], bf16, tag="qt")
            kt_sb = qkvpool.tile([P, T], bf16, tag="kt")
            vt_sb = qkvpool.tile([P, T], bf16, tag="vt")
            # V' per head: [k-part, ktile, 65]; col 64 = ones for row sums
            vp_sb = qkvpool.tile([P, 2, T // P, D + 1], bf16, tag="vp")
            nc.vector.memset(vp_sb[:, :, :, D : D + 1], 1.0)

            # PE warm-up while the first DMAs land: identity-only matmuls
            # ramp the HAM clock gate to full speed before real work. The
            # accumulated result is read once (into a V' slot that a later
            # vprime overwrites) so DCE keeps the chain.
            wu = psp.tile([P, P], f32, tag="pj", bufs=2, name="wups")
            for i in range(24):
                nc.tensor.matmul(
                    wu, ident_bf, ident_bf, start=(i == 0), stop=(i == 23)
                )
            nc.vector.tensor_copy(vp_sb[:, 0, 0, 0:D], wu[:, 0:D])

            def proj_group(t8, which):
                """Project 512 tokens (chunk t8) for q/k/v (which=0/1/2)."""
                w_sb, b_sb = w_sbs[which], b_sbs[which]
                dst = (qt_sb, kt_sb, vt_sb)[which]
                ps = psp.tile([P, 512], f32, tag="pj", bufs=2, name="projps")
                for a in range(HK):
                    nc.tensor.matmul(
                        ps,
                        w_sb[:, a, :],
                        xt_sb[:, a, ts(t8 % 4, 512)],
                        start=(a == 0),
                        stop=(a == HK - 1),
                    )
                nc.vector.tensor_scalar_add(dst[:, ts(t8, 512)], ps, b_sb)

            def vprime(head, kt32):
                """Transpose one [64,128] VT tile into V'[:, head, kt32]."""
                tp = psp.tile([P, D], bf16, tag="pj", bufs=2, name="vtps")
                nc.tensor.transpose(
                    tp,
                    vt_sb[ds(D * head, D), ts(kt32, P)],
                    ident_bf[ds(D * head, D), ds(D * head, D)],
                )
                nc.vector.tensor_copy(vp_sb[:, head, kt32, 0:D], tp)

            # PT ring: 2 units x 16 kt x [128, 1024] bf16 (128 KB/partition)
            RING = 32
            pt_all = ptpool.tile([P, 2, RING, 1024], bf16, tag="pt")

            def pv_and_norm(unit, head, j):
                """Deferred P@V + normalize for one (b, qh, head, j) quarter.

                Runs under a later unit's exp shadow: PE accumulates
                ctxT[d|sum, 512] over the 16 buffered PT tiles, then
                transposes, reciprocal-normalizes and DMAs out."""
                ctx = pv_acc(unit, head, j, 0, 16)
                pv_norm(ctx, unit, head, j)

            def pv_acc(unit, head, j, k0, k1, ctx=None):
                """P@V accumulation over buffered PT k-tiles [k0, k1)."""
                b = unit // 2
                if ctx is None:
                    ctx = psp.tile([D + 1, 512], f32, tag="ctx", bufs=2, name="ctx")
                for kt in range(k0, k1):
                    nc.tensor.matmul(
                        ctx,
                        vp_sb[:, head, b * 16 + kt, :],
                        pt_all[:, head, (unit * 16 + kt) % RING, ts(j, 512)],
                        start=(kt == 0),
                        stop=(kt == 15),
                    )
                return ctx

            def pv_norm(ctx, unit, head, j):
                qbase = (unit // 2) * S + (unit % 2) * 1024
                hb = D * head
                stg = stgpool.tile([D + 1, 512], f32, tag="stg")
                nc.vector.tensor_copy(stg, ctx)
                ot = otpool.tile([P, 4, D], f32, tag="ot")
                for tt in range(4):
                    tp = psp.tile([P, D + 1], f32, tag="pj", bufs=2, name="ntps")
                    nc.tensor.transpose(
                        tp, stg[:, ts(tt, P)], ident_f[0 : D + 1, 0 : D + 1]
                    )
                    rc = smallpool.tile([P, 1], f32, tag="rc")
                    nc.vector.reciprocal(rc, tp[:, D : D + 1])
                    nc.vector.tensor_scalar_mul(ot[:, tt, :], tp[:, 0:D], rc)
                dst = out[ds(qbase + j * 512, 512), ds(hb, D)].rearrange(
                    "(tt p) d -> p tt d", p=P
                )
                nc.sync.dma_start(dst, ot)

            # deferred-work queue: (cost, closure), drained between kt
            # Deferred-work queue: (cost, fn, deadline). Deadline (u, kt)
            # means the item MUST be emitted before (u, kt)'s scores/exp —
            # emission order is Tile's semantic order, so a late RAW
            # producer or a PT-ring WAR reader would read wrong data.
            # Items are popped by deadline (forced) or by cost pacing.
            work_q = deque()
            pv3_ctx = {}  # (head, j) -> open ctx accumulator for unit 3

            def q_proj(t8, which, dl):
                work_q.append((1.7, lambda: proj_group(t8, which), dl))

            def q_vp4(b, group, dl):  # 4 k-tiles x 2 heads
                for kk in range(4 * group, 4 * group + 4):
                    for head in range(2):
                        work_q.append(
                            (0.15, lambda h=head, k=kk: vprime(h, b * 16 + k), dl)
                        )

            def q_pv(unit, dl):
                for head in range(2):
                    for j in range(2):
                        work_q.append(
                            (
                                4.0,
                                lambda h=head, j=j: pv_and_norm(unit, h, j),
                                dl,
                            )
                        )

            NEVER = (9, 0)

            def push_unit_work(unit):
                if unit == 0:
                    # rest of batch 0 (essentials q0,q1,k0 already emitted)
                    q_proj(1, 1, (0, 4))  # k1
                    q_proj(0, 2, (1, 0))  # v0 (feeds V' -> pv(0) in unit 1)
                    q_vp4(0, 0, (1, 0))
                    q_proj(1, 2, (1, 0))
                    q_vp4(0, 1, (1, 0))
                    q_proj(2, 1, (0, 8))  # k2
                    q_proj(2, 2, (1, 0))
                    q_proj(2, 0, (1, 0))  # q2 (unit 1 scores)
                    q_vp4(0, 2, (1, 0))
                    q_proj(3, 1, (0, 12))  # k3
                    q_proj(3, 0, (1, 0))  # q3
                    q_proj(3, 2, (1, 0))
                    q_vp4(0, 3, (1, 0))
                elif unit == 1:
                    q_pv(0, (2, 0))  # PT slots reused by unit 2
                    q_proj(4, 1, (2, 0))  # k4
                    q_proj(4, 0, (2, 0))  # q4
                    q_proj(5, 0, (2, 0))  # q5
                    q_proj(4, 2, (3, 0))  # v4 + V' feed pv(2) in unit 3
                    q_vp4(1, 0, (3, 0))
                elif unit == 2:
                    q_proj(5, 1, (2, 4))  # k5
                    work_q.append((4.0, lambda: pv_and_norm(1, 0, 0), (3, 0)))
                    q_proj(6, 1, (2, 8))  # k6
                    work_q.append((4.0, lambda: pv_and_norm(1, 0, 1), (3, 0)))
                    q_proj(7, 1, (2, 12))  # k7
                    q_proj(6, 0, (3, 0))  # q6
                    q_proj(7, 0, (3, 0))  # q7
                    work_q.append((4.0, lambda: pv_and_norm(1, 1, 0), (3, 0)))
                    q_proj(5, 2, (3, 0))
                    q_vp4(1, 1, (3, 0))
                    work_q.append((4.0, lambda: pv_and_norm(1, 1, 1), (3, 0)))
                elif unit == 3:
                    # rest of batch 1's V' (feeds pv(2)/pv(3); FIFO keeps
                    # them ahead of the pv items), then pv(2)
                    q_proj(6, 2, NEVER)
                    q_vp4(1, 2, NEVER)
                    q_proj(7, 2, NEVER)
                    q_vp4(1, 3, NEVER)
                    q_pv(2, NEVER)

            # ---- batch 0 essentials: just enough for unit 0's scores ----
            proj_group(0, 0)  # q0
            proj_group(1, 0)  # q1
            proj_group(0, 1)  # k0

            for unit in range(4):
                b, qh = unit // 2, unit % 2
                base = b * S
                qbase = base + qh * 1024
                if unit == 1:
                    # drain every batch-0 consumer of xt_sb first: emission
                    # order is semantic order, so the reload must be emitted
                    # after all batch-0 projection reads
                    while work_q and work_q[0][2] <= (1, 0):
                        work_q.popleft()[1]()
                    # reload X^T with batch 1 tokens (WAR on batch-0 projs)
                    for quarter in range(4):
                        nc.sync.dma_start(
                            xt_sb[:, :, ts(quarter, 512)],
                            xtp[:, :, ds(S + quarter * 512, 512)],
                        )
                push_unit_work(unit)
                credit = 2.0
                for kt in range(16):
                    # forced pops: items whose emission-order deadline is due
                    while work_q and work_q[0][2] <= (unit, kt):
                        _, fn, _ = work_q.popleft()
                        fn()
                    sts = []
                    for head in range(2):
                        st = psp.tile(
                            [P, 1024], f32, tag="st", bufs=2, name=f"st{head}"
                        )
                        sts.append(st)
                    for j in range(2):
                        for head in range(2):
                            hb = D * head
                            nc.tensor.matmul(
                                sts[head][:, ts(j, 512)],
                                kt_sb[ds(hb, D), ds(base + kt * P, P)],
                                qt_sb[ds(hb, D), ds(qbase + j * 512, 512)],
                                start=True,
                                stop=True,
                            )
                    for head in range(2):
                        nc.scalar.activation(
                            pt_all[:, head, (unit * 16 + kt) % RING, :],
                            sts[head],
                            Exp,
                            scale=0.125,
                        )
                    if unit == 3 and kt == 8:
                        # last unit: first-half P@V accumulation can run
                        # under the remaining exp shadow (its PT tiles for
                        # kt 0-7 are final); only the second half + the
                        # normalize stay in the tail
                        for head in range(2):
                            for j in range(2):
                                work_q.append(
                                    (
                                        2.0,
                                        lambda h=head, j=j: pv3_ctx.__setitem__(
                                            (h, j), pv_acc(3, h, j, 0, 8)
                                        ),
                                        NEVER,
                                    )
                                )
                    # deferred work drained under the exp shadow, paced so
                    # the PE never runs far ahead of ACT
                    credit = min(credit + 1.4, 8.0)
                    while work_q and work_q[0][0] <= credit:
                        cost, fn, _ = work_q.popleft()
                        credit -= cost
                        fn()
            while work_q:
                work_q.popleft()[1]()
            for head in range(2):
                for j in range(2):
                    ctx = pv_acc(3, head, j, 8, 16, ctx=pv3_ctx[(head, j)])
                    pv_norm(ctx, 3, head, j)

    nc.compile()
    return nc


def get_nc():
    if "nc" not in _CACHED:
        _CACHED["nc"] = _build()
    return _CACHED["nc"]


def kernel(hidden_states, Wq, bq, Wk, bk, Wv, bv):
    from concourse.bass_utils import run_bass_kernel_spmd

    nc = get_nc()

    x2 = np.asarray(hidden_states, dtype=np.float32).reshape(T, HID)
    xt_b = np.ascontiguousarray(x2.T).astype(BF)

    in_maps = []
    for c in range(N_CORES):
        sl = slice(P * c, P * (c + 1))
        in_maps.append(
            {
                "xt": xt_b,
                "wq": np.ascontiguousarray(np.asarray(Wq, np.float32)[:, sl]).astype(BF),
                "wk": np.ascontiguousarray(np.asarray(Wk, np.float32)[:, sl]).astype(BF),
                "wv": np.ascontiguousarray(np.asarray(Wv, np.float32)[:, sl]).astype(BF),
                "bias": np.ascontiguousarray(
                    np.stack(
                        [
                            np.asarray(bq, np.float32)[sl],
                            np.asarray(bk, np.float32)[sl],
                            np.asarray(bv, np.float32)[sl],
                        ],
                        axis=1,
                    )
                ),
            }
        )

    res = run_bass_kernel_spmd(nc, in_maps, list(range(N_CORES)))

    full = np.empty((T, HID), dtype=np.float32)
    for c in range(N_CORES):
        full[:, P * c : P * (c + 1)] = res.results[c]["out"]
    return full.reshape(B, S, HID)
